# revision 18
# baseline (speedup 1.0000x reference)
"""Trainium2 Bass kernel for AssociativeIncrementalAttention.

Multi-head attention (B=2, S=2048, D=512, H=8, HD=64) with additive
[B,S,S] bias tensors, a concept-equality bias, and key-padding mask.

Sharding: 8 cores, fully data-parallel (no collectives).
  core c -> batch b = c//4, query rows q0 = (c%4)*512 .. q0+512.
Each core computes full K/V for its batch (replicated inside the
4-core batch group), scores for its 512 query rows, softmax via
exp(s/8)*exp(comb) with normalization deferred past the attn@V matmul
(rowsum rides the matmul through an appended ones-column in V).

Self-contained: hardcodes shapes; host-side prep is layout-only
(slices/transposes) plus tiny metadata casts (concept ids -> f32
sentinels, padding mask -> additive f32, position iotas).
"""

import sys

if "/opt/trn_rl_repo" not in sys.path:
    sys.path.insert(0, "/opt/trn_rl_repo")

import numpy as np

import concourse.bass as bass
import concourse.tile as tile
from concourse import bacc, mybir
from concourse import bass_utils

B, S, D, H = 2, 2048, 512, 8
HD = D // H  # 64
N_CORES = 8
QS = 512          # query rows per core
QT = QS // 128    # 4 query tiles per core
DC = D // 128     # 4 contraction chunks
SC512 = S // 512  # 4
SC128 = S // 128  # 16
F32 = mybir.dt.float32
F16 = mybir.dt.float16
BF16 = mybir.dt.bfloat16

_COMPILED = None


def _build():
    nc = bacc.Bacc("TRN2", target_bir_lowering=False, debug=False,
                   num_devices=N_CORES)

    def din(name, shape, dt=F32):
        return nc.dram_tensor(name, shape, dt, kind="ExternalInput").ap()

    xT = din("xT", [D, S])            # x[b].T
    xqT = din("xqT", [D, QS])         # x[b, q0:q0+QS].T
    wqT = din("wqT", [D, D])
    wkT = din("wkT", [D, D])
    wvT = din("wvT", [D, D])
    woT = din("woT", [D, D])
    bq = din("bq", [D])
    bk = din("bk", [D])
    bv = din("bv", [D])
    bo = din("bo", [D])
    ipaT = din("ipaT", [S, QS])
    ascT = din("ascT", [S, QS])
    mskT = din("mskT", [S, QS])
    cidq = din("cidq", [QS])
    qpos = din("qpos", [QS])
    cidkT = din("cidkT", [128, SC128])
    kpmT = din("kpmT", [128, SC128])
    kposT = din("kposT", [128, SC128])
    out = nc.dram_tensor("out", [QS, D], F32, kind="ExternalOutput").ap()

    AL = mybir.AluOpType
    AF = mybir.ActivationFunctionType

    def bcast_ap(src):
        # partition-broadcast read: [[0,128]] + original free dims
        return bass.AP(tensor=src.tensor, offset=src.offset,
                       ap=[[0, 128]] + list(src.ap))

    with tile.TileContext(nc) as tc:
        with (
            tc.tile_pool(name="persist", bufs=1) as P,
            tc.tile_pool(name="combwork", bufs=1) as CW,
            tc.tile_pool(name="pwork", bufs=3) as PW,
            tc.tile_pool(name="rswork", bufs=2) as RW,
            tc.tile_pool(name="osb", bufs=2) as OS,
            tc.tile_pool(name="spp", bufs=3, space="PSUM") as SPP,
            tc.tile_pool(name="ctxp", bufs=2, space="PSUM") as CP,
        ):
            # ---- persistent tiles ----
            kT_sb = P.tile([128, DC, S], BF16, tag="kT")
            qT_sb = P.tile([128, DC, QS], BF16, tag="qT")
            vplus = P.tile([128, SC128, 8 * 65], BF16, tag="vplus")
            vp4 = vplus.rearrange("p s (h c) -> p s h c", c=65)
            ebT = P.tile([128, SC128, QS], BF16, tag="ebT")
            ctxT_sb = P.tile([128, DC, QS], BF16, tag="ctxT")

            # ---- input loads; gpsimd issue order ~= arrival order ----
            w_sb = {}
            for nm, ap_ in (("wk", wkT), ("wq", wqT)):
                w_sb[nm] = P.tile([128, DC, D], BF16, tag=nm, name=nm)
                nc.gpsimd.dma_start(
                    out=w_sb[nm], in_=ap_.rearrange("(c p) s -> p c s", p=128))
            xqT_sb = P.tile([128, DC, QS], BF16, tag="xqT")
            nc.gpsimd.dma_start(
                out=xqT_sb, in_=xqT.rearrange("(c p) s -> p c s", p=128))
            cidqb = CW.tile([128, QS], BF16, tag="cidqb")
            nc.gpsimd.dma_start(out=cidqb, in_=bcast_ap(cidq))
            qposb = CW.tile([128, QS], F16, tag="qposb")
            nc.gpsimd.dma_start(out=qposb, in_=bcast_ap(qpos))
            xT_sb = P.tile([128, DC, S], BF16, tag="xT")
            for sc in range(2):
                nc.gpsimd.dma_start(
                    out=xT_sb[:, :, sc * 512:(sc + 1) * 512],
                    in_=xT.rearrange("(c p) s -> p c s", p=128)[
                        :, :, sc * 512:(sc + 1) * 512])
            # bias chunk 0 early so comb/ebT[0..3] is ready when head 0 runs
            bias_in = {}
            for nm, ap_ in (("ipa", ipaT), ("asc", ascT), ("msk", mskT)):
                bias_in[nm] = [None] * 4
            def load_bias_g4(g4, bufs=2):
                for nm, ap_ in (("ipa", ipaT), ("asc", ascT), ("msk", mskT)):
                    t = CW.tile([128, 4, QS], BF16, tag=nm, name=nm, bufs=bufs)
                    nc.gpsimd.dma_start(
                        out=t,
                        in_=ap_[g4 * 512:(g4 + 1) * 512, :].rearrange(
                            "(c p) s -> p c s", p=128))
                    bias_in[nm][g4] = t
            load_bias_g4(0)
            for sc in range(2, SC512):
                nc.gpsimd.dma_start(
                    out=xT_sb[:, :, sc * 512:(sc + 1) * 512],
                    in_=xT.rearrange("(c p) s -> p c s", p=128)[
                        :, :, sc * 512:(sc + 1) * 512])
            w_sb["wv"] = P.tile([128, DC, D], BF16, tag="wv", name="wv")
            nc.gpsimd.dma_start(
                out=w_sb["wv"], in_=wvT.rearrange("(c p) s -> p c s", p=128))
            load_bias_g4(1)
            w_sb["wo"] = P.tile([128, DC, D], BF16, tag="wo", name="wo")
            nc.gpsimd.dma_start(
                out=w_sb["wo"], in_=woT.rearrange("(c p) s -> p c s", p=128))
            wo_sb = w_sb["wo"]
            load_bias_g4(2)
            load_bias_g4(3)

            b_sb = {}
            for nm, ap_ in (("bq", bq), ("bk", bk), ("bv", bv)):
                b_sb[nm] = P.tile([128, DC], F32, tag=nm, name=nm)
                nc.sync.dma_start(out=b_sb[nm],
                                  in_=ap_.rearrange("(c p) -> p c", p=128))
            bo_row = P.tile([1, D], F32, tag="bo_row")
            nc.sync.dma_start(out=bo_row, in_=bo.rearrange("(a s) -> a s", a=1))
            cidkT_sb = P.tile([128, SC128], F32, tag="cidkT")
            nc.sync.dma_start(out=cidkT_sb, in_=cidkT)
            kpmT_sb = P.tile([128, SC128], F32, tag="kpmT")
            nc.sync.dma_start(out=kpmT_sb, in_=kpmT)
            kposT_sb = P.tile([128, SC128], F32, tag="kposT")
            nc.sync.dma_start(out=kposT_sb, in_=kposT)
            ones_row = P.tile([1, 128], BF16, tag="ones_row")
            nc.vector.memset(ones_row, 1.0)

            # ---- combined bias in [k, q] layout -> exp per 4-chunk ----
            # (DVE/ACT stream work; independent of PE projections)
            xfall = CW.tile([128, SC128, QS], BF16, tag="xfall")
            for g4 in range(4):
                ipa_t = bias_in["ipa"][g4]
                asc_t = bias_in["asc"][g4]
                msk_t = bias_in["msk"][g4]
                for j in range(4):
                    kc = g4 * 4 + j
                    ta = CW.tile([128, QS], BF16, tag="ta")
                    nc.vector.tensor_tensor(
                        out=ta, in0=ipa_t[:, j, :], in1=asc_t[:, j, :],
                        op=AL.add)
                    tb = CW.tile([128, QS], BF16, tag="tb")
                    nc.vector.tensor_tensor(
                        out=tb, in0=ta, in1=msk_t[:, j, :], op=AL.add)
                    npq05 = CW.tile([128, QS], BF16, tag="npq05")
                    nc.vector.tensor_scalar(
                        out=npq05, in0=qposb,
                        scalar1=kposT_sb[:, kc:kc + 1],
                        scalar2=0.5, op0=AL.not_equal, op1=AL.mult)
                    wcm = CW.tile([128, QS], BF16, tag="wcm")
                    nc.vector.scalar_tensor_tensor(
                        out=wcm, in0=cidqb,
                        scalar=cidkT_sb[:, kc:kc + 1], in1=npq05,
                        op0=AL.is_equal, op1=AL.mult)
                    nc.vector.scalar_tensor_tensor(
                        out=xfall[:, kc, :], in0=tb,
                        scalar=kpmT_sb[:, kc:kc + 1], in1=wcm,
                        op0=AL.add, op1=AL.add)
                nc.scalar.activation(
                    out=ebT[:, g4 * 4:(g4 + 1) * 4, :],
                    in_=xfall[:, g4 * 4:(g4 + 1) * 4, :], func=AF.Exp)

            # ---- projection helpers ----
            def proj_v():
                for scp in range(8):
                    ps = SPP.tile([128, 2, 512], F32, tag="sp", name="psv")
                    for i in range(2):
                        sc = scp * 2 + i
                        for dc in range(DC):
                            nc.tensor.matmul(
                                ps[:, i, :],
                                lhsT=xT_sb[:, dc, sc * 128:(sc + 1) * 128],
                                rhs=w_sb["wv"][:, dc, :],
                                start=(dc == 0), stop=(dc == DC - 1))
                    nc.scalar.copy(
                        out=vp4[:, scp * 2:(scp + 1) * 2, :, 0:64],
                        in_=ps.rearrange("p i (h c) -> p i h c", c=64))
                nc.vector.memset(vp4[:, :, :, 64:65], 1.0)

            def proj_k(oc):
                for scp in range(2):
                    ps = SPP.tile([128, 2, 512], F32, tag="sp", name="psk")
                    for i in range(2):
                        sc = scp * 2 + i
                        for dc in range(DC):
                            nc.tensor.matmul(
                                ps[:, i, :],
                                lhsT=w_sb["wk"][:, dc, oc * 128:(oc + 1) * 128],
                                rhs=xT_sb[:, dc, sc * 512:(sc + 1) * 512],
                                start=(dc == 0), stop=(dc == DC - 1))
                    nc.vector.tensor_scalar(
                        out=kT_sb[:, oc, scp * 1024:(scp + 1) * 1024],
                        in0=ps, scalar1=b_sb["bk"][:, oc:oc + 1],
                        scalar2=None, op0=AL.add)

            def proj_q(oc):
                ps = SPP.tile([128, 2, 512], F32, tag="sp", name="psq")
                for dc in range(DC):
                    nc.tensor.matmul(
                        ps[:, 0, :],
                        lhsT=w_sb["wq"][:, dc, oc * 128:(oc + 1) * 128],
                        rhs=xqT_sb[:, dc, :],
                        start=(dc == 0), stop=(dc == DC - 1))
                nc.vector.tensor_scalar(
                    out=qT_sb[:, oc, :], in0=ps[:, 0, :],
                    scalar1=b_sb["bq"][:, oc:oc + 1],
                    scalar2=None, op0=AL.add)

            def head(h):
                oc, rb = h // 2, (h % 2) * 64
                cps_h = CP.tile([65, QS], F32, tag="ctx", name="ctx")
                for g in range(8):
                    ps = SPP.tile([128, 2, 512], F32, tag="sp", name="pss")
                    for j in range(2):
                        kc = g * 2 + j
                        nc.tensor.matmul(
                            ps[:, j, :],
                            lhsT=kT_sb[rb:rb + 64, oc,
                                       kc * 128:(kc + 1) * 128],
                            rhs=qT_sb[rb:rb + 64, oc, :],
                            start=True, stop=True)
                    praw = PW.tile([128, 2, 512], BF16, tag="praw",
                                   name="praw")
                    nc.scalar.activation(out=praw, in_=ps, func=AF.Exp,
                                         scale=0.125)
                    pf = PW.tile([128, 2, 512], BF16, tag="pf", name="pf")
                    nc.vector.tensor_tensor(
                        out=pf, in0=praw,
                        in1=ebT[:, g * 2:(g + 1) * 2, :], op=AL.mult)
                    for j in range(2):
                        kc = g * 2 + j
                        nc.tensor.matmul(
                            cps_h,
                            lhsT=vplus[:, kc, h * 65:(h + 1) * 65],
                            rhs=pf[:, j, :],
                            start=(kc == 0), stop=(kc == SC128 - 1))
                rs_row = RW.tile([1, QS], F32, tag="rs_row", name="rs_row")
                nc.vector.tensor_copy(rs_row, cps_h[64:65, :])
                rr = RW.tile([1, QS], F32, tag="rr", name="rr")
                nc.vector.reciprocal_approx_fast(rr, rs_row)
                rrb = RW.tile([64, QS], F32, tag="rrb", name="rrb")
                nc.gpsimd.partition_broadcast(rrb, rr)
                nc.vector.tensor_tensor(
                    out=ctxT_sb[rb:rb + 64, oc, :],
                    in0=cps_h[0:64, :], in1=rrb, op=AL.mult)

            # ---- interleaved schedule: projections woven between heads ----
            proj_v()
            proj_q(0)
            proj_k(0)
            head(0)
            proj_k(1)
            proj_q(1)
            head(1)
            head(2)
            proj_k(2)
            proj_q(2)
            head(3)
            head(4)
            proj_k(3)
            proj_q(3)
            head(5)
            head(6)
            head(7)

            # cvec = Wo @ bv + bo  (rank-1 epilogue row)
            bv_bf = P.tile([128, DC], BF16, tag="bv_bf")
            nc.vector.tensor_copy(bv_bf, b_sb["bv"])
            cvec = P.tile([1, D], BF16, tag="cvec")
            cps = SPP.tile([128, 2, 512], F32, tag="sp", name="cps")
            for dc in range(DC):
                nc.tensor.matmul(cps[0:1, 0, :], lhsT=bv_bf[:, dc:dc + 1],
                                 rhs=wo_sb[:, dc, :],
                                 start=(dc == 0), stop=(dc == DC - 1))
            nc.vector.tensor_tensor(out=cvec, in0=cps[0:1, 0, :], in1=bo_row,
                                    op=AL.add)

            # ---- output projection ----
            for m in range(QT):
                pom = SPP.tile([128, 2, 512], F32, tag="sp", name="pom")
                for dc in range(DC):
                    nc.tensor.matmul(
                        pom[:, 0, :],
                        lhsT=ctxT_sb[:, dc, m * 128:(m + 1) * 128],
                        rhs=wo_sb[:, dc, :],
                        start=(dc == 0), stop=False)
                nc.tensor.matmul(pom[:, 0, :], lhsT=ones_row, rhs=cvec,
                                 start=False, stop=True)
                o_t = OS.tile([128, 512], F32, tag="o", name="o_t")
                nc.scalar.copy(out=o_t, in_=pom[:, 0, :])
                nc.sync.dma_start(out=out[m * 128:(m + 1) * 128, :],
                                  in_=o_t)

    nc.compile()
    return nc


def _prep_in_maps(inputs):
    x = np.asarray(inputs["x"], np.float32)
    ipa = np.asarray(inputs["ipa_affinity_bias"], np.float32)
    asc = np.asarray(inputs["assoc_bias"], np.float32)
    msk = np.asarray(inputs["attention_mask"], np.float32)
    cid = np.asarray(inputs["concept_ids"])
    kpm = np.asarray(inputs["key_padding_mask"])

    wT = {nm: np.ascontiguousarray(np.asarray(inputs[nm], np.float32).T)
          for nm in ("Wq", "Wk", "Wv", "Wo")}
    bias = {nm: np.asarray(inputs[nm], np.float32)
            for nm in ("bq", "bk", "bv", "bo")}

    xT = [np.ascontiguousarray(x[b].T) for b in range(B)]
    cidq_f = np.where(cid >= 0, cid, -1).astype(np.float32)
    cidk_f = np.where(cid >= 0, cid, -2).astype(np.float32)
    kpm_add = np.where(kpm, np.float32(-1e30), np.float32(0.0))
    kpos = np.arange(S, dtype=np.float32)

    in_maps = []
    for c in range(N_CORES):
        b, q0 = c // 4, (c % 4) * QS
        in_maps.append({
            "xT": xT[b],
            "xqT": np.ascontiguousarray(x[b, q0:q0 + QS].T),
            "wqT": wT["Wq"], "wkT": wT["Wk"],
            "wvT": wT["Wv"], "woT": wT["Wo"],
            "bq": bias["bq"], "bk": bias["bk"],
            "bv": bias["bv"], "bo": bias["bo"],
            "ipaT": np.ascontiguousarray(ipa[b, q0:q0 + QS].T),
            "ascT": np.ascontiguousarray(asc[b, q0:q0 + QS].T),
            "mskT": np.ascontiguousarray(msk[q0:q0 + QS].T),
            "cidq": np.ascontiguousarray(cidq_f[b, q0:q0 + QS]),
            "qpos": (q0 + np.arange(QS)).astype(np.float32),
            "cidkT": np.ascontiguousarray(cidk_f[b].reshape(SC128, 128).T),
            "kpmT": np.ascontiguousarray(kpm_add[b].reshape(SC128, 128).T),
            "kposT": np.ascontiguousarray(
                kpos.reshape(SC128, 128).T),
        })
    return in_maps


def run(inputs, trace=False):
    global _COMPILED
    if _COMPILED is None:
        _COMPILED = _build()
    nc = _COMPILED
    in_maps = _prep_in_maps(inputs)
    kw = {}
    if trace:
        kw = dict(trace=True, trace_cores=list(range(N_CORES)))
    res = bass_utils.run_bass_kernel_spmd(
        nc, in_maps, core_ids=list(range(N_CORES)), **kw)
    out = np.empty((B, S, D), np.float32)
    for c in range(N_CORES):
        b, q0 = c // 4, (c % 4) * QS
        out[b, q0:q0 + QS] = res.results[c]["out"]
    return out, res


def kernel(**inputs) -> np.ndarray:
    out, _ = run(inputs)
    return out


# revision 19
# speedup vs baseline: 1.0050x; 1.0050x over previous
"""Trainium2 Bass kernel for AssociativeIncrementalAttention.

Multi-head attention (B=2, S=2048, D=512, H=8, HD=64) with additive
[B,S,S] bias tensors, a concept-equality bias, and key-padding mask.

Sharding: 8 cores, fully data-parallel (no collectives).
  core c -> batch b = c//4, query rows q0 = (c%4)*512 .. q0+512.
Each core computes full K/V for its batch (replicated inside the
4-core batch group), scores for its 512 query rows, softmax via
exp(s/8)*exp(comb) with normalization deferred past the attn@V matmul
(rowsum rides the matmul through an appended ones-column in V).

Self-contained: hardcodes shapes; host-side prep is layout-only
(slices/transposes) plus tiny metadata casts (concept ids -> f32
sentinels, padding mask -> additive f32, position iotas).
"""

import sys

if "/opt/trn_rl_repo" not in sys.path:
    sys.path.insert(0, "/opt/trn_rl_repo")

import numpy as np

import concourse.bass as bass
import concourse.tile as tile
from concourse import bacc, mybir
from concourse import bass_utils

B, S, D, H = 2, 2048, 512, 8
HD = D // H  # 64
N_CORES = 8
QS = 512          # query rows per core
QT = QS // 128    # 4 query tiles per core
DC = D // 128     # 4 contraction chunks
SC512 = S // 512  # 4
SC128 = S // 128  # 16
F32 = mybir.dt.float32
F16 = mybir.dt.float16
BF16 = mybir.dt.bfloat16

_COMPILED = None


def _build():
    nc = bacc.Bacc("TRN2", target_bir_lowering=False, debug=False,
                   num_devices=N_CORES)

    def din(name, shape, dt=F32):
        return nc.dram_tensor(name, shape, dt, kind="ExternalInput").ap()

    xT = din("xT", [D, S])            # x[b].T
    xqT = din("xqT", [D, QS])         # x[b, q0:q0+QS].T
    wqT = din("wqT", [D, D])
    wkT = din("wkT", [D, D])
    wvT = din("wvT", [D, D])
    woT = din("woT", [D, D])
    bq = din("bq", [D])
    bk = din("bk", [D])
    bv = din("bv", [D])
    bo = din("bo", [D])
    ipaT = din("ipaT", [S, QS])
    ascT = din("ascT", [S, QS])
    mskT = din("mskT", [S, QS])
    cidq = din("cidq", [QS])
    qpos = din("qpos", [QS])
    cidkT = din("cidkT", [128, SC128])
    kpmT = din("kpmT", [128, SC128])
    kposT = din("kposT", [128, SC128])
    out = nc.dram_tensor("out", [QS, D], F32, kind="ExternalOutput").ap()

    AL = mybir.AluOpType
    AF = mybir.ActivationFunctionType

    def bcast_ap(src):
        # partition-broadcast read: [[0,128]] + original free dims
        return bass.AP(tensor=src.tensor, offset=src.offset,
                       ap=[[0, 128]] + list(src.ap))

    with tile.TileContext(nc) as tc:
        with (
            tc.tile_pool(name="persist", bufs=1) as P,
            tc.tile_pool(name="combwork", bufs=1) as CW,
            tc.tile_pool(name="pwork", bufs=3) as PW,
            tc.tile_pool(name="rswork", bufs=2) as RW,
            tc.tile_pool(name="osb", bufs=2) as OS,
            tc.tile_pool(name="spp", bufs=3, space="PSUM") as SPP,
            tc.tile_pool(name="ctxp", bufs=2, space="PSUM") as CP,
        ):
            # ---- persistent tiles ----
            kT_sb = P.tile([128, DC, S], BF16, tag="kT")
            qT_sb = P.tile([128, DC, QS], BF16, tag="qT")
            vplus = P.tile([128, SC128, 8 * 65], BF16, tag="vplus")
            vp4 = vplus.rearrange("p s (h c) -> p s h c", c=65)
            ebT = P.tile([128, SC128, QS], BF16, tag="ebT")
            ctxT_sb = P.tile([128, DC, QS], BF16, tag="ctxT")

            # ---- input loads; gpsimd issue order ~= arrival order ----
            w_sb = {}
            w_sb["wq"] = P.tile([128, DC, D], BF16, tag="wq", name="wq")
            nc.gpsimd.dma_start(
                out=w_sb["wq"], in_=wqT.rearrange("(c p) s -> p c s", p=128))
            xqT_sb = P.tile([128, DC, QS], BF16, tag="xqT")
            nc.gpsimd.dma_start(
                out=xqT_sb, in_=xqT.rearrange("(c p) s -> p c s", p=128))
            w_sb["wk"] = P.tile([128, DC, D], BF16, tag="wk", name="wk")
            nc.gpsimd.dma_start(
                out=w_sb["wk"], in_=wkT.rearrange("(c p) s -> p c s", p=128))
            xT_sb = P.tile([128, DC, S], BF16, tag="xT")
            for sc in range(SC512):
                nc.gpsimd.dma_start(
                    out=xT_sb[:, :, sc * 512:(sc + 1) * 512],
                    in_=xT.rearrange("(c p) s -> p c s", p=128)[
                        :, :, sc * 512:(sc + 1) * 512])
            w_sb["wv"] = P.tile([128, DC, D], BF16, tag="wv", name="wv")
            nc.gpsimd.dma_start(
                out=w_sb["wv"], in_=wvT.rearrange("(c p) s -> p c s", p=128))
            cidqb = CW.tile([128, QS], BF16, tag="cidqb")
            nc.gpsimd.dma_start(out=cidqb, in_=bcast_ap(cidq))
            qposb = CW.tile([128, QS], F16, tag="qposb")
            nc.gpsimd.dma_start(out=qposb, in_=bcast_ap(qpos))
            bias_in = {"ipa": [None] * 4, "asc": [None] * 4, "msk": [None] * 4}

            def load_bias_g4(g4, bufs=2):
                for nm, ap_ in (("ipa", ipaT), ("asc", ascT), ("msk", mskT)):
                    t = CW.tile([128, 4, QS], BF16, tag=nm, name=nm, bufs=bufs)
                    nc.gpsimd.dma_start(
                        out=t,
                        in_=ap_[g4 * 512:(g4 + 1) * 512, :].rearrange(
                            "(c p) s -> p c s", p=128))
                    bias_in[nm][g4] = t
            load_bias_g4(0)
            load_bias_g4(1)
            w_sb["wo"] = P.tile([128, DC, D], BF16, tag="wo", name="wo")
            nc.gpsimd.dma_start(
                out=w_sb["wo"], in_=woT.rearrange("(c p) s -> p c s", p=128))
            wo_sb = w_sb["wo"]
            load_bias_g4(2)
            load_bias_g4(3)

            b_sb = {}
            for nm, ap_ in (("bq", bq), ("bk", bk), ("bv", bv)):
                b_sb[nm] = P.tile([128, DC], F32, tag=nm, name=nm)
                nc.sync.dma_start(out=b_sb[nm],
                                  in_=ap_.rearrange("(c p) -> p c", p=128))
            bo_row = P.tile([1, D], F32, tag="bo_row")
            nc.sync.dma_start(out=bo_row, in_=bo.rearrange("(a s) -> a s", a=1))
            cidkT_sb = P.tile([128, SC128], F32, tag="cidkT")
            nc.sync.dma_start(out=cidkT_sb, in_=cidkT)
            kpmT_sb = P.tile([128, SC128], F32, tag="kpmT")
            nc.sync.dma_start(out=kpmT_sb, in_=kpmT)
            kposT_sb = P.tile([128, SC128], F32, tag="kposT")
            nc.sync.dma_start(out=kposT_sb, in_=kposT)
            ones_row = P.tile([1, 128], BF16, tag="ones_row")
            nc.vector.memset(ones_row, 1.0)

            # ---- combined bias in [k, q] layout -> exp per 4-chunk ----
            # (DVE/ACT stream work; independent of PE projections)
            xfall = CW.tile([128, SC128, QS], BF16, tag="xfall")
            for g4 in range(4):
                ipa_t = bias_in["ipa"][g4]
                asc_t = bias_in["asc"][g4]
                msk_t = bias_in["msk"][g4]
                for j in range(4):
                    kc = g4 * 4 + j
                    ta = CW.tile([128, QS], BF16, tag="ta")
                    nc.vector.tensor_tensor(
                        out=ta, in0=ipa_t[:, j, :], in1=asc_t[:, j, :],
                        op=AL.add)
                    tb = CW.tile([128, QS], BF16, tag="tb")
                    nc.vector.tensor_tensor(
                        out=tb, in0=ta, in1=msk_t[:, j, :], op=AL.add)
                    npq05 = CW.tile([128, QS], BF16, tag="npq05")
                    nc.vector.tensor_scalar(
                        out=npq05, in0=qposb,
                        scalar1=kposT_sb[:, kc:kc + 1],
                        scalar2=0.5, op0=AL.not_equal, op1=AL.mult)
                    wcm = CW.tile([128, QS], BF16, tag="wcm")
                    nc.vector.scalar_tensor_tensor(
                        out=wcm, in0=cidqb,
                        scalar=cidkT_sb[:, kc:kc + 1], in1=npq05,
                        op0=AL.is_equal, op1=AL.mult)
                    nc.vector.scalar_tensor_tensor(
                        out=xfall[:, kc, :], in0=tb,
                        scalar=kpmT_sb[:, kc:kc + 1], in1=wcm,
                        op0=AL.add, op1=AL.add)
                nc.scalar.activation(
                    out=ebT[:, g4 * 4:(g4 + 1) * 4, :],
                    in_=xfall[:, g4 * 4:(g4 + 1) * 4, :], func=AF.Exp)

            # ---- projection helpers ----
            def proj_v():
                for scp in range(8):
                    ps = SPP.tile([128, 2, 512], F32, tag="sp", name="psv")
                    for i in range(2):
                        sc = scp * 2 + i
                        for dc in range(DC):
                            nc.tensor.matmul(
                                ps[:, i, :],
                                lhsT=xT_sb[:, dc, sc * 128:(sc + 1) * 128],
                                rhs=w_sb["wv"][:, dc, :],
                                start=(dc == 0), stop=(dc == DC - 1))
                    nc.scalar.copy(
                        out=vp4[:, scp * 2:(scp + 1) * 2, :, 0:64],
                        in_=ps.rearrange("p i (h c) -> p i h c", c=64))
                nc.vector.memset(vp4[:, :, :, 64:65], 1.0)

            def proj_k(oc):
                for scp in range(2):
                    ps = SPP.tile([128, 2, 512], F32, tag="sp", name="psk")
                    for i in range(2):
                        sc = scp * 2 + i
                        for dc in range(DC):
                            nc.tensor.matmul(
                                ps[:, i, :],
                                lhsT=w_sb["wk"][:, dc, oc * 128:(oc + 1) * 128],
                                rhs=xT_sb[:, dc, sc * 512:(sc + 1) * 512],
                                start=(dc == 0), stop=(dc == DC - 1))
                    nc.vector.tensor_scalar(
                        out=kT_sb[:, oc, scp * 1024:(scp + 1) * 1024],
                        in0=ps, scalar1=b_sb["bk"][:, oc:oc + 1],
                        scalar2=None, op0=AL.add)

            def proj_q(oc):
                ps = SPP.tile([128, 2, 512], F32, tag="sp", name="psq")
                for dc in range(DC):
                    nc.tensor.matmul(
                        ps[:, 0, :],
                        lhsT=w_sb["wq"][:, dc, oc * 128:(oc + 1) * 128],
                        rhs=xqT_sb[:, dc, :],
                        start=(dc == 0), stop=(dc == DC - 1))
                nc.vector.tensor_scalar(
                    out=qT_sb[:, oc, :], in0=ps[:, 0, :],
                    scalar1=b_sb["bq"][:, oc:oc + 1],
                    scalar2=None, op0=AL.add)

            def head(h):
                oc, rb = h // 2, (h % 2) * 64
                cps_h = CP.tile([65, QS], F32, tag="ctx", name="ctx")
                for g in range(8):
                    ps = SPP.tile([128, 2, 512], F32, tag="sp", name="pss")
                    for j in range(2):
                        kc = g * 2 + j
                        nc.tensor.matmul(
                            ps[:, j, :],
                            lhsT=kT_sb[rb:rb + 64, oc,
                                       kc * 128:(kc + 1) * 128],
                            rhs=qT_sb[rb:rb + 64, oc, :],
                            start=True, stop=True)
                    praw = PW.tile([128, 2, 512], BF16, tag="praw",
                                   name="praw")
                    nc.scalar.activation(out=praw, in_=ps, func=AF.Exp,
                                         scale=0.125)
                    pf = PW.tile([128, 2, 512], BF16, tag="pf", name="pf")
                    nc.vector.tensor_tensor(
                        out=pf, in0=praw,
                        in1=ebT[:, g * 2:(g + 1) * 2, :], op=AL.mult)
                    for j in range(2):
                        kc = g * 2 + j
                        nc.tensor.matmul(
                            cps_h,
                            lhsT=vplus[:, kc, h * 65:(h + 1) * 65],
                            rhs=pf[:, j, :],
                            start=(kc == 0), stop=(kc == SC128 - 1))
                rs_row = RW.tile([1, QS], F32, tag="rs_row", name="rs_row")
                nc.vector.tensor_copy(rs_row, cps_h[64:65, :])
                rr = RW.tile([1, QS], F32, tag="rr", name="rr")
                nc.vector.reciprocal_approx_fast(rr, rs_row)
                rrb = RW.tile([64, QS], F32, tag="rrb", name="rrb")
                nc.gpsimd.partition_broadcast(rrb, rr)
                nc.vector.tensor_tensor(
                    out=ctxT_sb[rb:rb + 64, oc, :],
                    in0=cps_h[0:64, :], in1=rrb, op=AL.mult)

            # ---- interleaved schedule: projections woven between heads ----
            proj_q(0)
            proj_k(0)
            proj_v()
            head(0)
            proj_k(1)
            proj_q(1)
            head(1)
            head(2)
            proj_k(2)
            proj_q(2)
            head(3)
            head(4)
            proj_k(3)
            proj_q(3)
            head(5)
            head(6)
            head(7)

            # cvec = Wo @ bv + bo  (rank-1 epilogue row)
            bv_bf = P.tile([128, DC], BF16, tag="bv_bf")
            nc.vector.tensor_copy(bv_bf, b_sb["bv"])
            cvec = P.tile([1, D], BF16, tag="cvec")
            cps = SPP.tile([128, 2, 512], F32, tag="sp", name="cps")
            for dc in range(DC):
                nc.tensor.matmul(cps[0:1, 0, :], lhsT=bv_bf[:, dc:dc + 1],
                                 rhs=wo_sb[:, dc, :],
                                 start=(dc == 0), stop=(dc == DC - 1))
            nc.vector.tensor_tensor(out=cvec, in0=cps[0:1, 0, :], in1=bo_row,
                                    op=AL.add)

            # ---- output projection ----
            for m in range(QT):
                pom = SPP.tile([128, 2, 512], F32, tag="sp", name="pom")
                for dc in range(DC):
                    nc.tensor.matmul(
                        pom[:, 0, :],
                        lhsT=ctxT_sb[:, dc, m * 128:(m + 1) * 128],
                        rhs=wo_sb[:, dc, :],
                        start=(dc == 0), stop=False)
                nc.tensor.matmul(pom[:, 0, :], lhsT=ones_row, rhs=cvec,
                                 start=False, stop=True)
                o_t = OS.tile([128, 512], F32, tag="o", name="o_t")
                nc.scalar.copy(out=o_t, in_=pom[:, 0, :])
                nc.sync.dma_start(out=out[m * 128:(m + 1) * 128, :],
                                  in_=o_t)

    nc.compile()
    return nc


def _prep_in_maps(inputs):
    x = np.asarray(inputs["x"], np.float32)
    ipa = np.asarray(inputs["ipa_affinity_bias"], np.float32)
    asc = np.asarray(inputs["assoc_bias"], np.float32)
    msk = np.asarray(inputs["attention_mask"], np.float32)
    cid = np.asarray(inputs["concept_ids"])
    kpm = np.asarray(inputs["key_padding_mask"])

    wT = {nm: np.ascontiguousarray(np.asarray(inputs[nm], np.float32).T)
          for nm in ("Wq", "Wk", "Wv", "Wo")}
    bias = {nm: np.asarray(inputs[nm], np.float32)
            for nm in ("bq", "bk", "bv", "bo")}

    xT = [np.ascontiguousarray(x[b].T) for b in range(B)]
    cidq_f = np.where(cid >= 0, cid, -1).astype(np.float32)
    cidk_f = np.where(cid >= 0, cid, -2).astype(np.float32)
    kpm_add = np.where(kpm, np.float32(-1e30), np.float32(0.0))
    kpos = np.arange(S, dtype=np.float32)

    in_maps = []
    for c in range(N_CORES):
        b, q0 = c // 4, (c % 4) * QS
        in_maps.append({
            "xT": xT[b],
            "xqT": np.ascontiguousarray(x[b, q0:q0 + QS].T),
            "wqT": wT["Wq"], "wkT": wT["Wk"],
            "wvT": wT["Wv"], "woT": wT["Wo"],
            "bq": bias["bq"], "bk": bias["bk"],
            "bv": bias["bv"], "bo": bias["bo"],
            "ipaT": np.ascontiguousarray(ipa[b, q0:q0 + QS].T),
            "ascT": np.ascontiguousarray(asc[b, q0:q0 + QS].T),
            "mskT": np.ascontiguousarray(msk[q0:q0 + QS].T),
            "cidq": np.ascontiguousarray(cidq_f[b, q0:q0 + QS]),
            "qpos": (q0 + np.arange(QS)).astype(np.float32),
            "cidkT": np.ascontiguousarray(cidk_f[b].reshape(SC128, 128).T),
            "kpmT": np.ascontiguousarray(kpm_add[b].reshape(SC128, 128).T),
            "kposT": np.ascontiguousarray(
                kpos.reshape(SC128, 128).T),
        })
    return in_maps


def run(inputs, trace=False):
    global _COMPILED
    if _COMPILED is None:
        _COMPILED = _build()
    nc = _COMPILED
    in_maps = _prep_in_maps(inputs)
    kw = {}
    if trace:
        kw = dict(trace=True, trace_cores=list(range(N_CORES)))
    res = bass_utils.run_bass_kernel_spmd(
        nc, in_maps, core_ids=list(range(N_CORES)), **kw)
    out = np.empty((B, S, D), np.float32)
    for c in range(N_CORES):
        b, q0 = c // 4, (c % 4) * QS
        out[b, q0:q0 + QS] = res.results[c]["out"]
    return out, res


def kernel(**inputs) -> np.ndarray:
    out, _ = run(inputs)
    return out
